# revision 4
# baseline (speedup 1.0000x reference)
import math
import sys

sys.path.insert(0, "/opt/trn_rl_repo")

import numpy as np

N_CORES = 8
B, T, D = 65536, 64, 10
B_CORE = B // N_CORES  # 8192
P128 = 128

_cache = {}


def build(Pv, sharpv, Lv, b_core=B_CORE, tb=16):
    """Build + compile the per-core SPMD Bass program.

    Math: s_t = x_t + y_t, carry c_t (c_0 = 0), u_t = s_t + c_t:
      c_{t+1} = sigmoid(sharp*(u_t - 9.5))
      logits[t,d] = L*cos((2pi/P)*(u_t - d))

    On-chip representation: h_t = tanh(sharp*(u_t-9.5)/2) = 2*c_{t+1}-1
    (Tanh and Sin share one ACT table set; Sigmoid and Sin do not), and
    the angle tile Z = delta*(u - 9) + gamma where delta = 2pi/P and
    gamma = 9.5*delta + pi/2, i.e. Z = delta*u + pi/2 = the d=0 sin angle:
      Z_t = (delta/2)*h_{t-1} + [delta*(s_t - 9) + gamma]     (h_{-1} = -1)
      h_t = tanh(Z_t/(2*alpha_inv) ...) = Tanh(scale=P/(4pi)*sharp^-1...)
    logits[t,d]   = L*sin(wrap(Z_t - delta*d))   for d in [0,5)
    logits[t,d+5] = -logits[t,d]                 (since 5*delta = pi exactly
                                                  when P = 10)
    where wrap() reduces into [-pi, pi] via ADD_RANGE_WRAP custom DVE ops.
    """
    import concourse.bacc as bacc
    import concourse.mybir as mybir
    import concourse.tile as tile

    fp32 = mybir.dt.float32
    i32 = mybir.dt.int32
    A = mybir.ActivationFunctionType
    Alu = mybir.AluOpType
    TWO_PI = 2.0 * math.pi
    NF = b_core // P128
    assert Pv == 10.0, "d+5 sign trick assumes P == 10"

    delta = TWO_PI / Pv                      # angle per digit unit
    gamma = 9.5 * delta + math.pi / 2.0      # Z = delta*u + pi/2
    # tanh argument: z/2 = sharp*(u-9.5)/2 = (Z - gamma)*sharp/(2*delta)
    th_scale = sharpv / (2.0 * delta)
    th_bias = -gamma * th_scale

    nc = bacc.Bacc(
        "TRN2", target_bir_lowering=False, debug=False, num_devices=N_CORES
    )
    x_d = nc.dram_tensor("x_dram", [b_core, T], i32, kind="ExternalInput").ap()
    y_d = nc.dram_tensor("y_dram", [b_core, T], i32, kind="ExternalInput").ap()
    lg_d = nc.dram_tensor(
        "logits_dram", [b_core, T, D], fp32, kind="ExternalOutput"
    ).ap()
    cr_d = nc.dram_tensor("carry_dram", [b_core], fp32, kind="ExternalOutput").ap()

    xv = x_d.rearrange("(p n) t -> p (n t)", p=P128)
    yv = y_d.rearrange("(p n) t -> p (n t)", p=P128)
    lv = lg_d.rearrange("(p n) t d -> p n t d", p=P128)
    cv = cr_d.rearrange("(p n) -> p n", p=P128)

    with tile.TileContext(nc) as tc:
        with (
            tc.tile_pool(name="main", bufs=1) as mp,
            tc.tile_pool(name="hp", bufs=2) as hp,
            tc.tile_pool(name="lp", bufs=2) as lp,
            tc.tile_pool(name="tp", bufs=3) as tp,
        ):
            xs = mp.tile([P128, NF * T], i32, tag="xs")
            ys = mp.tile([P128, NF * T], i32, tag="ys")
            nc.sync.dma_start(xs[:], xv)
            nc.sync.dma_start(ys[:], yv)

            # ssum = x + y (as fp32); zs = delta*ssum + (gamma - 9*delta)
            ssum = mp.tile([P128, NF * T], fp32, tag="scratch")
            nc.vector.tensor_tensor(ssum[:], xs[:], ys[:], Alu.add)
            zs = mp.tile([P128, NF * T], fp32, tag="zs")
            nc.vector.tensor_scalar(
                zs[:], ssum[:], float(delta), float(gamma - 9.0 * delta),
                Alu.mult, Alu.add,
            )
            zs3 = zs[:].rearrange("p (n t) -> p n t", t=T)

            Z = mp.tile([P128, NF * T], fp32, tag="Z")
            Z3 = Z[:].rearrange("p (n t) -> p n t", t=T)

            # tanh bias const AP
            thb = mp.tile([P128, 1], fp32, tag="thb")
            nc.vector.memset(thb[:], float(th_bias))

            h_prev = hp.tile([P128, NF], fp32, tag="h")
            nc.vector.memset(h_prev[:], -1.0)
            for t in range(T):
                nc.vector.scalar_tensor_tensor(
                    Z3[:, :, t],
                    h_prev[:],
                    float(delta * 0.5),
                    zs3[:, :, t],
                    Alu.mult,
                    Alu.add,
                )
                h_new = hp.tile([P128, NF], fp32, tag="h")
                nc.scalar.activation(
                    h_new[:], Z3[:, :, t], A.Tanh, bias=thb[:], scale=float(th_scale)
                )
                h_prev = h_new

            cfin = hp.tile([P128, NF], fp32, tag="cfin")
            nc.vector.tensor_scalar(cfin[:], h_prev[:], 0.5, 0.5, Alu.mult, Alu.add)
            nc.sync.dma_start(cv, cfin[:])

            # base range reduction: w0 = wrap(Z - 2pi) in [-pi, pi], == Z mod 2pi
            w0 = mp.tile([P128, NF * T], fp32, tag="scratch")
            nc.vector.add_range_wrap(
                w0[:], Z[:], float(-TWO_PI), float(math.pi), float(TWO_PI)
            )
            w03 = w0[:].rearrange("p (n t) -> p n t", t=T)

            nblk = T // tb
            for bi in range(nblk):
                b0 = bi * tb
                Lt = lp.tile([P128, NF * tb * D], fp32, tag="L")
                L4 = Lt[:].rearrange("p (n t d) -> p n t d", t=tb, d=D)
                for d in range(5):
                    if d == 0:
                        wd_ap = w03[:, :, b0 : b0 + tb]
                    else:
                        wd = tp.tile([P128, NF * tb], fp32, tag="wd")
                        nc.vector.add_range_wrap(
                            wd[:],
                            w03[:, :, b0 : b0 + tb],
                            float(-delta * d),
                            float(math.pi),
                            float(TWO_PI),
                        )
                        wd_ap = wd[:]
                    sd = tp.tile([P128, NF * tb], fp32, tag="sd")
                    nc.scalar.activation(sd[:], wd_ap, A.Sin, bias=0.0, scale=1.0)
                    sd3 = sd[:].rearrange("p (n t) -> p n t", t=tb)
                    # scatter * (+L) into d, * (-L) into d+5; split DVE/POOL
                    nc.gpsimd.tensor_scalar_mul(L4[:, :, :, d], sd3, float(Lv))
                    eng = nc.vector if d % 2 == 0 else nc.gpsimd
                    eng.tensor_scalar_mul(L4[:, :, :, d + 5], sd3, float(-Lv))
                nc.sync.dma_start(lv[:, :, b0 : b0 + tb, :], L4)

    nc.compile()
    return nc


def kernel(x_digits_rev, y_digits_rev, P, sharp, logit_scale):
    from concourse import bass_utils

    x = np.ascontiguousarray(np.asarray(x_digits_rev), dtype=np.int32)
    y = np.ascontiguousarray(np.asarray(y_digits_rev), dtype=np.int32)
    Pv = float(np.asarray(P))
    sv = float(np.asarray(sharp))
    Lv = float(np.asarray(logit_scale))
    key = (Pv, sv, Lv)
    if key not in _cache:
        _cache[key] = build(Pv, sv, Lv)
    nc = _cache[key]
    in_maps = [
        {
            "x_dram": np.ascontiguousarray(x[c * B_CORE : (c + 1) * B_CORE]),
            "y_dram": np.ascontiguousarray(y[c * B_CORE : (c + 1) * B_CORE]),
        }
        for c in range(N_CORES)
    ]
    res = bass_utils.run_bass_kernel_spmd(nc, in_maps, core_ids=list(range(N_CORES)))
    logits = np.concatenate(
        [res.results[c]["logits_dram"] for c in range(N_CORES)], axis=0
    )
    carry = np.concatenate(
        [res.results[c]["carry_dram"] for c in range(N_CORES)], axis=0
    )
    return logits, carry
